# revision 15
# baseline (speedup 1.0000x reference)
"""Conv2d(128->256, 3x3, pad=1) over (32,128,56,56), data-parallel across 8
NeuronCores (4 images per core).

Per core: conv = 9 shifted accumulating matmuls per output tile.
  - contraction K = Cin = 128 (partition dim)
  - stationary lhsT = W^T[ci, co_tile] per (ky,kx)  -> [128, 128] bf16
  - moving rhs = input pixels [128, <=8 rows, <=56 cols] (N <= 448)
  - PSUM accumulates the 9 (ky,kx) taps; padding handled by clipping each
    tap's matmul to the valid rectangle (center tap goes first with
    start=True and covers the full tile, so partial-range taps accumulate
    on top via PSUM's per-element has_written bits).
Bias is added during the PSUM->SBUF copy (ScalarE/VectorE alternating).

Latency structure:
  - first image is loaded in row-quarters on the Sync HWDGE ring while the
    weights load in cot-halves on the Scalar HWDGE ring, so the first
    matmul can start as soon as quarter 0 + weight half 0 land;
  - a handful of zero dummy matmuls bridge the PE from the preamble to the
    first data-dependent matmul so the HAM clock-gate warms early;
  - images 1..3 prefetch on the GpSimd SWDGE queue;
  - output stores go out in row-quarters alternating Sync/Scalar rings so
    the final store before the exit barrier is small.
"""

import numpy as np
import ml_dtypes

import concourse.mybir as mybir
import concourse.tile as tile
from concourse import bacc
from concourse.bass_utils import run_bass_kernel_spmd

N_CORES = 8
B, CIN, H, W = 32, 128, 56, 56
COUT, R, S = 256, 3, 3
BL = B // N_CORES          # images per core
NCOT = COUT // 128         # Cout tiles of 128
YCHUNK = 8                 # output rows per matmul tile
NYC = H // YCHUNK

MM_DT = mybir.dt.bfloat16
MM_NP = ml_dtypes.bfloat16

NWARM = 5                  # dummy matmuls to bridge PE from preamble to data
X0_SPLITS = [0, 16, 32, 44, 56]       # first-image load quarters (rows)
OUT_SPLITS = {1: (0, 14), 3: (14, 28), 5: (28, 42), 6: (42, 56)}  # yc -> store rows
# tap order in the weight layout: center tap first so its DMA lands first
TAP_ORDER = [(1, 1), (0, 0), (0, 1), (0, 2), (1, 0), (1, 2), (2, 0), (2, 1), (2, 2)]
TAP_IDX = {t: i for i, t in enumerate(TAP_ORDER)}

_cache = {}


def _build():
    if "nc" in _cache:
        return _cache["nc"]
    nc = bacc.Bacc("TRN2", target_bir_lowering=False, debug=False)
    f32 = mybir.dt.float32
    x_d = nc.dram_tensor("x", [BL, CIN, H, W], MM_DT, kind="ExternalInput").ap()
    w_d = nc.dram_tensor("w", [CIN, NCOT, R * S, 128], MM_DT, kind="ExternalInput").ap()
    b_d = nc.dram_tensor("b", [128, NCOT], f32, kind="ExternalInput").ap()
    y_d = nc.dram_tensor("y", [BL, COUT, H, W], f32, kind="ExternalOutput").ap()

    with tile.TileContext(nc) as tc:
        with (
            tc.tile_pool(name="consts", bufs=1) as cpool,
            tc.tile_pool(name="xin", bufs=2) as xpool,
            tc.tile_pool(name="yout", bufs=2) as opool,
            tc.tile_pool(name="ps", bufs=8, space="PSUM") as pspool,
        ):
            # --- PE prewarm: zero matmuls with no DMA dependency ---
            warm_x = cpool.tile([128, 512], MM_DT)
            nc.vector.memset(warm_x[:], 0.0)
            warm_ps = pspool.tile([128, 512], f32, tag="ps")
            for _ in range(NWARM):
                nc.tensor.matmul(
                    warm_ps[:], warm_x[:, 0:128], warm_x[:], start=True, stop=True
                )

            # --- constants + first image, on parallel HWDGE rings ---
            # center tap of cot 0 first: it gates the very first data matmul
            w_sb = cpool.tile([CIN, NCOT, R * S, 128], MM_DT)
            nc.scalar.dma_start(w_sb[:, 0, 0:1], w_d[:, 0, 0:1])
            nc.scalar.dma_start(w_sb[:, 0, 1:], w_d[:, 0, 1:])
            nc.scalar.dma_start(w_sb[:, 1], w_d[:, 1])
            b_sb = cpool.tile([128, NCOT], f32)
            nc.gpsimd.dma_start(b_sb[:], b_d[:])

            x_tiles = []
            x0 = xpool.tile([CIN, H, W], MM_DT, name="x_sb_0", tag="x_sb")
            for r0, r1 in zip(X0_SPLITS, X0_SPLITS[1:]):
                nc.sync.dma_start(x0[:, r0:r1, :], x_d[0, :, r0:r1, :])
            x_tiles.append(x0)

            for img in range(BL):
                if img > 0:
                    x_sb = xpool.tile(
                        [CIN, H, W], MM_DT, name=f"x_sb_{img}", tag="x_sb"
                    )
                    nc.gpsimd.dma_start(x_sb[:], x_d[img])
                else:
                    x_sb = x_tiles[0]
                for cot in range(NCOT):
                    o_sb = opool.tile(
                        [128, H, W], f32, name=f"o_sb_{img}_{cot}", tag="o_sb"
                    )
                    for yc in range(NYC):
                        y0 = YCHUNK * yc
                        ps = pspool.tile(
                            [128, YCHUNK, W], f32, name=f"ps_{img}_{cot}_{yc}", tag="ps"
                        )
                        # center tap first: full-tile write with start=True
                        nc.tensor.matmul(
                            ps[:],
                            w_sb[:, cot, 0, :],
                            x_sb[:, y0 : y0 + YCHUNK, :],
                            start=True,
                            stop=False,
                        )
                        for ti, (ky, kx) in enumerate(TAP_ORDER[1:], start=1):
                            oy0 = max(0, 1 - ky - y0)
                            oy1 = min(YCHUNK, H + 1 - y0 - ky)
                            ox0 = max(0, 1 - kx)
                            ox1 = min(W, W + 1 - kx)
                            nc.tensor.matmul(
                                ps[:, oy0:oy1, ox0:ox1],
                                w_sb[:, cot, ti, :],
                                x_sb[
                                    :,
                                    y0 + oy0 + ky - 1 : y0 + oy1 + ky - 1,
                                    ox0 + kx - 1 : ox1 + kx - 1,
                                ],
                                start=False,
                                stop=(ti == R * S - 1),
                            )
                        # PSUM -> SBUF with fused bias add; alternate engines
                        if yc % 2 == 0:
                            nc.scalar.activation(
                                o_sb[:, y0 : y0 + YCHUNK, :],
                                ps[:],
                                mybir.ActivationFunctionType.Identity,
                                bias=b_sb[:, cot : cot + 1],
                            )
                        else:
                            nc.vector.tensor_scalar_add(
                                o_sb[:, y0 : y0 + YCHUNK, :],
                                ps[:],
                                b_sb[:, cot : cot + 1],
                            )
                        # store finished row-quarters, alternating HWDGE rings
                        if yc in OUT_SPLITS:
                            r0, r1 = OUT_SPLITS[yc]
                            q = list(OUT_SPLITS).index(yc)
                            eng = nc.sync if (img + cot + q) % 2 == 0 else nc.scalar
                            last = img == BL - 1 and cot == NCOT - 1 and yc == NYC - 1
                            if last:
                                # split the final store so the exit barrier
                                # waits on a small transfer
                                rm = (r0 + r1) // 2
                                nc.sync.dma_start(
                                    y_d[img, 128 * cot : 128 * (cot + 1), r0:rm, :],
                                    o_sb[:, r0:rm, :],
                                )
                                nc.scalar.dma_start(
                                    y_d[img, 128 * cot : 128 * (cot + 1), rm:r1, :],
                                    o_sb[:, rm:r1, :],
                                )
                            else:
                                eng.dma_start(
                                    y_d[img, 128 * cot : 128 * (cot + 1), r0:r1, :],
                                    o_sb[:, r0:r1, :],
                                )

    nc.compile()
    _cache["nc"] = nc
    return nc


def _in_maps(inputs, weight, bias):
    x = np.asarray(inputs).astype(MM_NP)
    # weight (co, ci, ky, kx) -> (ci, cot, tap, co_in_tile), taps in TAP_ORDER
    wt = (
        np.asarray(weight)
        .reshape(NCOT, 128, CIN, R, S)
        .transpose(2, 0, 3, 4, 1)  # (ci, cot, ky, kx, co)
        .astype(MM_NP)
    )
    w = np.ascontiguousarray(
        np.stack([wt[:, :, ky, kx, :] for ky, kx in TAP_ORDER], axis=2)
    )
    b = np.ascontiguousarray(
        np.asarray(bias).astype(np.float32).reshape(NCOT, 128).T
    )
    return [
        {"x": np.ascontiguousarray(x[c * BL : (c + 1) * BL]), "w": w, "b": b}
        for c in range(N_CORES)
    ]


def kernel(inputs, weight, bias):
    nc = _build()
    in_maps = _in_maps(inputs, weight, bias)
    res = run_bass_kernel_spmd(nc, in_maps, core_ids=list(range(N_CORES)))
    return np.concatenate([res.results[c]["y"] for c in range(N_CORES)], axis=0)


# revision 17
# speedup vs baseline: 1.0664x; 1.0664x over previous
"""Conv2d(128->256, 3x3, pad=1) over (32,128,56,56), data-parallel across 8
NeuronCores (4 images per core).

Per core: conv = 9 shifted accumulating matmuls per output tile.
  - contraction K = Cin = 128 (partition dim)
  - stationary lhsT = W^T[ci, co_tile] per (ky,kx)  -> [128, 128] bf16
  - moving rhs = input pixels [128, <=8 rows, <=56 cols] (N <= 448)
  - PSUM accumulates the 9 (ky,kx) taps; padding handled by clipping each
    tap's matmul to the valid rectangle (center tap goes first with
    start=True and covers the full tile, so partial-range taps accumulate
    on top via PSUM's per-element has_written bits).
Bias is added during the PSUM->SBUF copy (ScalarE/VectorE alternating).

Latency structure:
  - first image is loaded in row-quarters on the Sync HWDGE ring while the
    weights load in cot-halves on the Scalar HWDGE ring, so the first
    matmul can start as soon as quarter 0 + weight half 0 land;
  - a handful of zero dummy matmuls bridge the PE from the preamble to the
    first data-dependent matmul so the HAM clock-gate warms early;
  - images 1..3 prefetch on the GpSimd SWDGE queue;
  - output stores go out in row-quarters alternating Sync/Scalar rings so
    the final store before the exit barrier is small.
"""

import numpy as np
import ml_dtypes

import concourse.mybir as mybir
import concourse.tile as tile
from concourse import bacc
from concourse.bass_utils import run_bass_kernel_spmd

N_CORES = 8
B, CIN, H, W = 32, 128, 56, 56
COUT, R, S = 256, 3, 3
BL = B // N_CORES          # images per core
NCOT = COUT // 128         # Cout tiles of 128
YCHUNK = 8                 # output rows per matmul tile
NYC = H // YCHUNK

MM_DT = mybir.dt.bfloat16
MM_NP = ml_dtypes.bfloat16

NWARM = 6                  # dummy matmuls to bridge PE from preamble to data
X0_SPLITS = [0, 16, 32, 44, 56]       # first-image load quarters (rows)
OUT_SPLITS = {1: (0, 14), 3: (14, 28), 5: (28, 42), 6: (42, 56)}  # yc -> store rows
# tap order in the weight layout: center tap first so its DMA lands first
TAP_ORDER = [(1, 1), (0, 0), (0, 1), (0, 2), (1, 0), (1, 2), (2, 0), (2, 1), (2, 2)]
TAP_IDX = {t: i for i, t in enumerate(TAP_ORDER)}

_cache = {}


def _build():
    if "nc" in _cache:
        return _cache["nc"]
    nc = bacc.Bacc("TRN2", target_bir_lowering=False, debug=False)
    f32 = mybir.dt.float32
    x_d = nc.dram_tensor("x", [BL, CIN, H, W], MM_DT, kind="ExternalInput").ap()
    w_d = nc.dram_tensor("w", [CIN, NCOT, R * S, 128], MM_DT, kind="ExternalInput").ap()
    b_d = nc.dram_tensor("b", [128, NCOT], f32, kind="ExternalInput").ap()
    y_d = nc.dram_tensor("y", [BL, COUT, H, W], f32, kind="ExternalOutput").ap()

    with tile.TileContext(nc) as tc:
        with (
            tc.tile_pool(name="consts", bufs=1) as cpool,
            tc.tile_pool(name="xin", bufs=2) as xpool,
            tc.tile_pool(name="yout", bufs=2) as opool,
            tc.tile_pool(name="ps", bufs=8, space="PSUM") as pspool,
        ):
            # --- PE prewarm: zero matmuls with no DMA dependency ---
            warm_x = cpool.tile([128, 512], MM_DT)
            nc.vector.memset(warm_x[:], 0.0)
            warm_ps = pspool.tile([128, 512], f32, tag="ps")
            for _ in range(NWARM):
                nc.tensor.matmul(
                    warm_ps[:], warm_x[:, 0:128], warm_x[:], start=True, stop=True
                )

            # --- constants + first image, on parallel HWDGE rings ---
            # one DMA per cot half: per-DMA fixed latency dominates at these
            # sizes, so finer splits arrive LATER (measured)
            w_sb = cpool.tile([CIN, NCOT, R * S, 128], MM_DT)
            nc.scalar.dma_start(w_sb[:, 0], w_d[:, 0])
            nc.scalar.dma_start(w_sb[:, 1], w_d[:, 1])
            b_sb = cpool.tile([128, NCOT], f32)
            nc.gpsimd.dma_start(b_sb[:], b_d[:])

            x_tiles = []
            x0 = xpool.tile([CIN, H, W], MM_DT, name="x_sb_0", tag="x_sb")
            for r0, r1 in zip(X0_SPLITS, X0_SPLITS[1:]):
                nc.sync.dma_start(x0[:, r0:r1, :], x_d[0, :, r0:r1, :])
            x_tiles.append(x0)

            for img in range(BL):
                if img > 0:
                    x_sb = xpool.tile(
                        [CIN, H, W], MM_DT, name=f"x_sb_{img}", tag="x_sb"
                    )
                    nc.gpsimd.dma_start(x_sb[:], x_d[img])
                else:
                    x_sb = x_tiles[0]
                for cot in range(NCOT):
                    o_sb = opool.tile(
                        [128, H, W], f32, name=f"o_sb_{img}_{cot}", tag="o_sb"
                    )
                    for yc in range(NYC):
                        y0 = YCHUNK * yc
                        ps = pspool.tile(
                            [128, YCHUNK, W], f32, name=f"ps_{img}_{cot}_{yc}", tag="ps"
                        )
                        # center tap first: full-tile write with start=True
                        nc.tensor.matmul(
                            ps[:],
                            w_sb[:, cot, 0, :],
                            x_sb[:, y0 : y0 + YCHUNK, :],
                            start=True,
                            stop=False,
                        )
                        for ti, (ky, kx) in enumerate(TAP_ORDER[1:], start=1):
                            oy0 = max(0, 1 - ky - y0)
                            oy1 = min(YCHUNK, H + 1 - y0 - ky)
                            ox0 = max(0, 1 - kx)
                            ox1 = min(W, W + 1 - kx)
                            nc.tensor.matmul(
                                ps[:, oy0:oy1, ox0:ox1],
                                w_sb[:, cot, ti, :],
                                x_sb[
                                    :,
                                    y0 + oy0 + ky - 1 : y0 + oy1 + ky - 1,
                                    ox0 + kx - 1 : ox1 + kx - 1,
                                ],
                                start=False,
                                stop=(ti == R * S - 1),
                            )
                        # PSUM -> SBUF with fused bias add; alternate engines
                        if yc % 2 == 0:
                            nc.scalar.activation(
                                o_sb[:, y0 : y0 + YCHUNK, :],
                                ps[:],
                                mybir.ActivationFunctionType.Identity,
                                bias=b_sb[:, cot : cot + 1],
                            )
                        else:
                            nc.vector.tensor_scalar_add(
                                o_sb[:, y0 : y0 + YCHUNK, :],
                                ps[:],
                                b_sb[:, cot : cot + 1],
                            )
                        # store finished row-quarters, alternating HWDGE rings
                        if yc in OUT_SPLITS:
                            r0, r1 = OUT_SPLITS[yc]
                            q = list(OUT_SPLITS).index(yc)
                            eng = nc.sync if (img + cot + q) % 2 == 0 else nc.scalar
                            last = img == BL - 1 and cot == NCOT - 1 and yc == NYC - 1
                            if last:
                                # split the final store so the exit barrier
                                # waits on a small transfer
                                rm = (r0 + r1) // 2
                                nc.sync.dma_start(
                                    y_d[img, 128 * cot : 128 * (cot + 1), r0:rm, :],
                                    o_sb[:, r0:rm, :],
                                )
                                nc.scalar.dma_start(
                                    y_d[img, 128 * cot : 128 * (cot + 1), rm:r1, :],
                                    o_sb[:, rm:r1, :],
                                )
                            else:
                                eng.dma_start(
                                    y_d[img, 128 * cot : 128 * (cot + 1), r0:r1, :],
                                    o_sb[:, r0:r1, :],
                                )

    nc.compile()
    _cache["nc"] = nc
    return nc


def _in_maps(inputs, weight, bias):
    x = np.asarray(inputs).astype(MM_NP)
    # weight (co, ci, ky, kx) -> (ci, cot, tap, co_in_tile), taps in TAP_ORDER
    wt = (
        np.asarray(weight)
        .reshape(NCOT, 128, CIN, R, S)
        .transpose(2, 0, 3, 4, 1)  # (ci, cot, ky, kx, co)
        .astype(MM_NP)
    )
    w = np.ascontiguousarray(
        np.stack([wt[:, :, ky, kx, :] for ky, kx in TAP_ORDER], axis=2)
    )
    b = np.ascontiguousarray(
        np.asarray(bias).astype(np.float32).reshape(NCOT, 128).T
    )
    return [
        {"x": np.ascontiguousarray(x[c * BL : (c + 1) * BL]), "w": w, "b": b}
        for c in range(N_CORES)
    ]


def kernel(inputs, weight, bias):
    nc = _build()
    in_maps = _in_maps(inputs, weight, bias)
    res = run_bass_kernel_spmd(nc, in_maps, core_ids=list(range(N_CORES)))
    return np.concatenate([res.results[c]["y"] for c in range(N_CORES)], axis=0)
